# revision 13
# baseline (speedup 1.0000x reference)
"""nn_GRUBlock Trainium2 kernel: y = GRU2(gelu(GRU1(x))).

Layer-pipelined rewrite of the weight-stationary baseline:
  - L1 chunk k and L2 chunk k-1 run with steps interleaved in one For_i
    body, so each layer's gate-chain latency hides under the other
    layer's PE matmuls (serial rounds: 8192 -> ~4096).
  - b_hh_n enters the n-gate PSUM group via K=1 bias matmuls (removes a
    DVE op from the serial chain); rz biases ride in xp via the GEMM
    copy (ACT Identity with per-partition bias).
  - xp double-buffered per layer so next-chunk GEMMs overlap recurrence.
  - GEMM PSUM tiles padded to a full bank (zero-region isolation).

Sharding: batch 16 -> 8 cores x NB=2, full inputs in / full output out.
"""

from contextlib import ExitStack

import numpy as np

B, T, DIN, H = 16, 4096, 512, 512
N_CORES = 8
NB = B // N_CORES
S = 128                # chunk length (steps)
U = 16                 # For_i unroll
NCH = T // S           # chunks per layer

_CACHE = {}


def _build():
    import concourse.bacc as bacc
    import concourse.bass as bass
    import concourse.tile as tile
    from concourse import mybir

    F32 = mybir.dt.float32
    F16 = mybir.dt.float16
    AF = mybir.ActivationFunctionType
    ALU = mybir.AluOpType

    nc = bacc.Bacc("TRN2", target_bir_lowering=False, debug=False,
                   enable_asserts=False)

    SN = S * NB

    xT = nc.dram_tensor("xT", [512, T * NB], F16, kind="ExternalInput").ap()
    wih1 = nc.dram_tensor("wih1", [512, 12 * 128], F16, kind="ExternalInput").ap()
    whh1 = nc.dram_tensor("whh1", [512, 12 * 128], F16, kind="ExternalInput").ap()
    bias1 = nc.dram_tensor("bias1", [128, 12], F32, kind="ExternalInput").ap()
    bhhn1 = nc.dram_tensor("bhhn1", [128, 8], F16, kind="ExternalInput").ap()
    wih2 = nc.dram_tensor("wih2", [512, 12 * 128], F16, kind="ExternalInput").ap()
    whh2 = nc.dram_tensor("whh2", [512, 12 * 128], F16, kind="ExternalInput").ap()
    bias2 = nc.dram_tensor("bias2", [128, 12], F32, kind="ExternalInput").ap()
    bhhn2 = nc.dram_tensor("bhhn2", [128, 8], F16, kind="ExternalInput").ap()
    eyed = nc.dram_tensor("eyed", [128, 128], F16, kind="ExternalInput").ap()
    y = nc.dram_tensor("y", [128, T * 4 * NB], F16, kind="ExternalOutput").ap()
    y4 = y.rearrange("p (t j b) -> p t j b", j=4, b=NB)

    with tile.TileContext(nc) as tc, ExitStack() as ctx:
        sb = ctx.enter_context(tc.tile_pool(name="sb", bufs=1))
        psp = ctx.enter_context(tc.tile_pool(name="psp", bufs=1, space="PSUM"))

        # ---- static tiles -------------------------------------------------
        def load_w(dram, name):
            t = sb.tile([128, 4 * 12 * 128], F16, name=name, tag=name)
            for j in range(4):
                nc.sync.dma_start(t[:, j * 12 * 128:(j + 1) * 12 * 128],
                                  dram[j * 128:(j + 1) * 128, :])
            return t

        wih_sb = [load_w(wih1, "wih1sb"), load_w(wih2, "wih2sb")]
        whh_sb = [load_w(whh1, "whh1sb"), load_w(whh2, "whh2sb")]

        def load_small(dram, name, rows, w, dt):
            t = sb.tile([128, w], dt, name=name, tag=name)
            nc.sync.dma_start(t[0:rows, :], dram[:])
            return t

        bias_sb = [load_small(bias1, "bias1sb", 128, 12, F32),
                   load_small(bias2, "bias2sb", 128, 12, F32)]
        bhhn_sb = [load_small(bhhn1, "bhhn1sb", 128, 8, F16),
                   load_small(bhhn2, "bhhn2sb", 128, 8, F16)]
        ones2 = sb.tile([128, 2], F16, name="ones2", tag="ones2")
        nc.vector.memset(ones2[0:1, :], 1.0)
        half_t = sb.tile([128, 1], F32, name="half_t", tag="half_t")
        nc.vector.memset(half_t[:], 0.5)
        eye128 = sb.tile([128, 128], F16, name="eye128", tag="eye128")
        nc.sync.dma_start(eye128[:], eyed[:])

        # ---- per-layer state ---------------------------------------------
        # step psum: one full bank per (layer, parity)
        psS = [[psp.tile([128, 512], F32, name=f"psS_{l}_{p}", tag=f"psS_{l}_{p}")
                for p in range(2)] for l in range(2)]
        # gemm psum: one full bank per (layer, parity)
        psG = [[psp.tile([128, 512], F32, name=f"psG_{l}_{p}", tag=f"psG_{l}_{p}")
                for p in range(2)] for l in range(2)]

        # xp buffers [128, 12, S, NB] fp16, parity per chunk
        xp = [[sb.tile([128, 12 * SN], F16, name=f"xp_{l}_{p}", tag=f"xp_{l}_{p}")
               for p in range(2)] for l in range(2)]
        xp4 = [[xp[l][p].rearrange("p (m t b) -> p m t b", m=12, b=NB)
                for p in range(2)] for l in range(2)]

        # h slabs [128, (S+1), 4, NB] fp16
        co = [sb.tile([128, (S + 1) * 4 * NB], F16, name=f"co_{l}", tag=f"co_{l}")
              for l in range(2)]
        co4 = [co[l].rearrange("p (t j b) -> p t j b", j=4, b=NB)
               for l in range(2)]
        # fixed-address h ping-pong (matmul operands must not use register
        # offsets -- each register AP costs an extra ~100ns Tensor op and
        # serializes the mm stream at ~171ns/pair instead of ~33ns)
        hfix = [[sb.tile([128, 4 * NB], F16, name=f"hfix_{l}_{p}",
                         tag=f"hfix_{l}_{p}") for p in range(2)]
                for l in range(2)]
        hfix4 = [[hfix[l][p].rearrange("p (j b) -> p j b", b=NB)
                  for p in range(2)] for l in range(2)]
        # fixed-address xp prefetch, fp16 (feeds the identity-matmul psum
        # injection and the t2 add; copy depends only on the chunk GEMM)
        xcur = [[sb.tile([128, 24], F16, name=f"xcur_{l}_{p}",
                         tag=f"xcur_{l}_{p}") for p in range(2)]
                for l in range(2)]

        # x input chunks (fp16), parity
        xin = [sb.tile([128, 4 * SN], F16, name=f"xin_{p}", tag=f"xin_{p}")
               for p in range(2)]
        # mid (gelu output) chunks [128, 4, S*NB] fp16, parity
        mid = [sb.tile([128, 4 * SN], F16, name=f"mid_{p}", tag=f"mid_{p}")
               for p in range(2)]
        mid4 = [mid[p].rearrange("p (j t b) -> p j t b", j=4, b=NB)
                for p in range(2)]

        # gate scratch: ONE set shared by both layers and all rounds. The
        # WAR hazards (each op's write waits the previous round/layer's last
        # reader) pin the Tile scheduler to the staggered round-robin order
        # on every engine queue -- without this it phase-groups ops by
        # dependency depth and a blocked op head-of-line-stalls ready work.
        def t(nm, w):
            return sb.tile([128, w], F32, name=nm, tag=nm)
        # sg is per-layer: breaks the cross-stream WAR (sigma_X waiting the
        # other stream's late ee read) that delayed sigma ~0.6us per step.
        # The rest stays shared to keep the round-robin pinning.
        sg = [t("sg0", 16), t("sg1", 16)]
        scratch = (t("t1", 8), t("t2", 8),
                   t("nn", 8), t("zz", 8), t("zh", 8), t("ee", 8))

        # gelu scratch (chunk-wide)
        erf_t = sb.tile([128, S * 4 * NB], F32, name="erf_t", tag="erf_t")
        gu = sb.tile([128, S * 4 * NB], F32, name="gu", tag="gu")

        nc.vector.memset(hfix[0][0][:], 0.0)
        nc.vector.memset(hfix[1][0][:], 0.0)

        # ---- helpers ------------------------------------------------------
        def dma_xin(k):
            t = xin[k % 2]
            for j in range(4):
                nc.sync.dma_start(t[:, j * SN:(j + 1) * SN],
                                  xT[j * 128:(j + 1) * 128, k * SN:(k + 1) * SN])

        def gemm(l, k, src_of_j):
            """xp[l][k%2] = wih_l^T @ src + bias (ACT copy adds bias)."""
            x4 = xp4[l][k % 2]
            for m in range(12):
                ps = psG[l][m % 2]
                for j in range(4):
                    nc.tensor.matmul(
                        ps[:, 0:SN],
                        wih_sb[l][:, (j * 12 + m) * 128:(j * 12 + m + 1) * 128],
                        src_of_j(j), start=(j == 0), stop=(j == 3))
                ps_v = ps[:, 0:SN].rearrange("p (t b) -> p t b", b=NB)
                nc.vector.tensor_scalar_add(x4[:, m, :, :], ps_v,
                                            bias_sb[l][:, m:m + 1])

        def step(l, i, par, co4l, x4):
            """One GRU step for layer l at time i, psum/h parity par."""
            P = psS[l][par]
            P4 = P[:, 0:24].rearrange("p (m b) -> p m b", b=NB)
            hc = hfix4[l][par]
            hnew = hfix[l][1 - par]
            xc = xcur[l][par]
            nc.vector.tensor_copy(xc[:], x4[:, :, bass.ds(i, 1), :])
            w = whh_sb[l]
            # xp_rz and b_hh_n enter the PSUM group via identity matmuls
            # (start=True pends the whole bank; every later mm accumulates)
            nc.tensor.matmul(P[:, 0:16], eye128[:, :], xc[:, 0:16],
                             start=True, stop=False)
            nc.tensor.matmul(P[:, 16:24], eye128[:, :], bhhn_sb[l][:, :],
                             start=False, stop=False)
            for m in range(12):
                for j in range(4):
                    nc.tensor.matmul(
                        P4[:, m, :],
                        w[:, (j * 12 + m) * 128:(j * 12 + m + 1) * 128],
                        hc[:, j, :], start=False,
                        stop=(m == 11 and j == 3))
            u1, u2, nnt, zz, zh, e = scratch
            sgt = sg[l]
            # sigmoid gives [r | zc=1-z] (z weights pre-negated on host)
            nc.scalar.activation(sgt[:], P[:, 0:16], AF.Sigmoid)
            # off the critical path: z = 1-zc, zh = z*h_old
            nc.gpsimd.tensor_scalar(zz[:], sgt[:, 8:16], -1.0, 1.0,
                                    ALU.mult, ALU.add)
            nc.gpsimd.tensor_mul(zh[:], zz[:], hfix[l][par][:])
            # critical chain: u1 -> u2 -> tanh -> ee -> h'
            nc.vector.tensor_mul(u1[:], sgt[:, 0:8], P[:, 16:24])
            nc.vector.tensor_add(u2[:], u1[:], xc[:, 16:24])
            nc.scalar.activation(nnt[:], u2[:], AF.Tanh)
            nc.gpsimd.tensor_mul(e[:], sgt[:, 8:16], nnt[:])
            nc.gpsimd.tensor_add(hnew[:], e[:], zh[:])
            nc.gpsimd.tensor_copy(co4l[:, bass.ds(i + 1, 1), :, :], hnew[:])

        def run_steps(layers):
            """layers: list of (l, xp-parity). Interleave steps in For_i."""
            with tc.For_i(0, S, U) as iv:
                for u in range(U):
                    i = iv + u
                    for (l, xpar) in layers:
                        step(l, i, u % 2, co4[l], xp4[l][xpar])

        def gelu(k):
            src = co4[0][:, 1:S + 1, :, :]
            nc.scalar.activation(erf_t[:], src, AF.Erf, scale=0.7071067811865476)
            nc.scalar.activation(gu[:], erf_t[:], AF.Identity,
                                 bias=half_t[:, 0:1], scale=0.5)
            m4 = mid4[k % 2]
            out_ap = m4.rearrange("p j t b -> p t j b")
            nc.vector.tensor_mul(out_ap, src, gu[:].rearrange(
                "p (t j b) -> p t j b", j=4, b=NB))

        def dma_y(k):
            nc.sync.dma_start(y4[:, k * S:(k + 1) * S, :, :],
                              co4[1][:, 1:S + 1, :, :])

        # ---- schedule -----------------------------------------------------
        # prologue: chunk 0 of L1 alone
        dma_xin(0)
        gemm(0, 0, lambda j: xin[0][:, j * SN:(j + 1) * SN])
        dma_xin(1)
        run_steps([(0, 0)])
        gelu(0)
        gemm(0, 1, lambda j: xin[1][:, j * SN:(j + 1) * SN])
        gemm(1, 0, lambda j, _k=0: mid4[0][:, j, :, :])

        for k in range(1, NCH):
            if k + 1 < NCH:
                dma_xin(k + 1)
            run_steps([(0, k % 2), (1, (k - 1) % 2)])
            dma_y(k - 1)
            gelu(k)
            if k + 1 < NCH:
                gemm(0, k + 1,
                     lambda j, _p=(k + 1) % 2: xin[_p][:, j * SN:(j + 1) * SN])
            gemm(1, k, lambda j, _p=k % 2: mid4[_p][:, j, :, :])

        # epilogue: last chunk of L2 alone
        run_steps([(1, (NCH - 1) % 2)])
        dma_y(NCH - 1)

    nc.compile()
    return nc


def _get_nc():
    if "nc" not in _CACHE:
        _CACHE["nc"] = _build()
    return _CACHE["nc"]


def _prep_core_inputs(x_slice, w_ih1, w_hh1, b_ih1, b_hh1,
                      w_ih2, w_hh2, b_ih2, b_hh2):
    def wstat(w):
        # [3H, D] -> [D, 3H], z block (cols H:2H) negated so sigmoid(rz
        # psum) yields [r, 1-z] in one op.
        wt = np.ascontiguousarray(w.T).astype(np.float64)
        wt[:, H:2 * H] = -wt[:, H:2 * H]
        return wt.astype(np.float16)

    def bias12(b_ih, b_hh):
        b = b_ih.astype(np.float64).copy()
        b[:2 * H] += b_hh[:2 * H].astype(np.float64)
        b[H:2 * H] = -b[H:2 * H]
        # n gates: b_ih only (b_hh_n injected per step via bias matmuls)
        return np.ascontiguousarray(b.reshape(12, 128).T).astype(np.float32)

    def bhhn(b_hh):
        bn = b_hh[2 * H:].reshape(4, 128).T
        return np.ascontiguousarray(
            np.repeat(bn[:, :, None], NB, axis=2).reshape(128, 4 * NB)
        ).astype(np.float16)

    xT = np.ascontiguousarray(
        x_slice.transpose(2, 1, 0).reshape(512, T * NB)).astype(np.float16)
    return {
        "xT": xT,
        "wih1": wstat(w_ih1), "whh1": wstat(w_hh1),
        "bias1": bias12(b_ih1, b_hh1), "bhhn1": bhhn(b_hh1),
        "wih2": wstat(w_ih2), "whh2": wstat(w_hh2),
        "bias2": bias12(b_ih2, b_hh2), "bhhn2": bhhn(b_hh2),
        "eyed": np.eye(128, dtype=np.float16),
    }


def kernel(x, w_ih1, w_hh1, b_ih1, b_hh1, w_ih2, w_hh2, b_ih2, b_hh2):
    from concourse import bass_utils

    x = np.asarray(x, dtype=np.float32)
    args = [np.asarray(a, dtype=np.float32) for a in
            (w_ih1, w_hh1, b_ih1, b_hh1, w_ih2, w_hh2, b_ih2, b_hh2)]

    nc = _get_nc()
    in_maps = [
        _prep_core_inputs(x[c * NB:(c + 1) * NB], *args)
        for c in range(N_CORES)
    ]
    res = bass_utils.run_bass_kernel_spmd(nc, in_maps,
                                          core_ids=list(range(N_CORES)))
    parts = []
    for c in range(N_CORES):
        yf = res.results[c]["y"].astype(np.float32).reshape(128, T, 4, NB)
        parts.append(np.ascontiguousarray(
            yf.transpose(3, 1, 2, 0).reshape(NB, T, 512)))
    return np.concatenate(parts, axis=0)



# revision 14
# speedup vs baseline: 1.4039x; 1.4039x over previous
"""nn_GRUBlock Trainium2 kernel: y = GRU2(gelu(GRU1(x))).

Layer-pipelined rewrite of the weight-stationary baseline:
  - L1 chunk k and L2 chunk k-1 run with steps interleaved in one For_i
    body, so each layer's gate-chain latency hides under the other
    layer's PE matmuls (serial rounds: 8192 -> ~4096).
  - b_hh_n enters the n-gate PSUM group via K=1 bias matmuls (removes a
    DVE op from the serial chain); rz biases ride in xp via the GEMM
    copy (ACT Identity with per-partition bias).
  - xp double-buffered per layer so next-chunk GEMMs overlap recurrence.
  - GEMM PSUM tiles padded to a full bank (zero-region isolation).

Sharding: batch 16 -> 8 cores x NB=2, full inputs in / full output out.
"""

from contextlib import ExitStack

import numpy as np

B, T, DIN, H = 16, 4096, 512, 512
N_CORES = 8
NB = B // N_CORES
S = 128                # chunk length (steps)
U = 16                 # For_i unroll
NCH = T // S           # chunks per layer

_CACHE = {}


def _build():
    import concourse.bacc as bacc
    import concourse.bass as bass
    import concourse.tile as tile
    from concourse import mybir

    F32 = mybir.dt.float32
    F16 = mybir.dt.float16
    AF = mybir.ActivationFunctionType

    nc = bacc.Bacc("TRN2", target_bir_lowering=False, debug=False,
                   enable_asserts=False)

    SN = S * NB

    xT = nc.dram_tensor("xT", [512, T * NB], F16, kind="ExternalInput").ap()
    wih1 = nc.dram_tensor("wih1", [512, 12 * 128], F16, kind="ExternalInput").ap()
    whh1 = nc.dram_tensor("whh1", [512, 12 * 128], F16, kind="ExternalInput").ap()
    bias1 = nc.dram_tensor("bias1", [128, 12], F32, kind="ExternalInput").ap()
    bhhn1 = nc.dram_tensor("bhhn1", [128, 8], F16, kind="ExternalInput").ap()
    wih2 = nc.dram_tensor("wih2", [512, 12 * 128], F16, kind="ExternalInput").ap()
    whh2 = nc.dram_tensor("whh2", [512, 12 * 128], F16, kind="ExternalInput").ap()
    bias2 = nc.dram_tensor("bias2", [128, 12], F32, kind="ExternalInput").ap()
    bhhn2 = nc.dram_tensor("bhhn2", [128, 8], F16, kind="ExternalInput").ap()
    eyed = nc.dram_tensor("eyed", [128, 128], F16, kind="ExternalInput").ap()
    y = nc.dram_tensor("y", [128, T * 4 * NB], F16, kind="ExternalOutput").ap()
    y4 = y.rearrange("p (t j b) -> p t j b", j=4, b=NB)

    with tile.TileContext(nc) as tc, ExitStack() as ctx:
        sb = ctx.enter_context(tc.tile_pool(name="sb", bufs=1))
        psp = ctx.enter_context(tc.tile_pool(name="psp", bufs=1, space="PSUM"))

        # ---- static tiles -------------------------------------------------
        def load_w(dram, name):
            t = sb.tile([128, 4 * 12 * 128], F16, name=name, tag=name)
            for j in range(4):
                nc.sync.dma_start(t[:, j * 12 * 128:(j + 1) * 12 * 128],
                                  dram[j * 128:(j + 1) * 128, :])
            return t

        wih_sb = [load_w(wih1, "wih1sb"), load_w(wih2, "wih2sb")]
        whh_sb = [load_w(whh1, "whh1sb"), load_w(whh2, "whh2sb")]

        def load_small(dram, name, rows, w, dt):
            t = sb.tile([128, w], dt, name=name, tag=name)
            nc.sync.dma_start(t[0:rows, :], dram[:])
            return t

        bias_sb = [load_small(bias1, "bias1sb", 128, 12, F32),
                   load_small(bias2, "bias2sb", 128, 12, F32)]
        bhhn_sb = [load_small(bhhn1, "bhhn1sb", 128, 8, F16),
                   load_small(bhhn2, "bhhn2sb", 128, 8, F16)]
        ones2 = sb.tile([128, 2], F16, name="ones2", tag="ones2")
        nc.vector.memset(ones2[0:1, :], 1.0)
        half_t = sb.tile([128, 1], F32, name="half_t", tag="half_t")
        nc.vector.memset(half_t[:], 0.5)
        eye128 = sb.tile([128, 128], F16, name="eye128", tag="eye128")
        nc.sync.dma_start(eye128[:], eyed[:])

        # ---- per-layer state ---------------------------------------------
        # step psum: one full bank per (layer, parity)
        psS = [[psp.tile([128, 512], F32, name=f"psS_{l}_{p}", tag=f"psS_{l}_{p}")
                for p in range(2)] for l in range(2)]
        # gemm psum: one full bank per (layer, parity)
        psG = [[psp.tile([128, 512], F32, name=f"psG_{l}_{p}", tag=f"psG_{l}_{p}")
                for p in range(2)] for l in range(2)]

        # xp buffers [128, 12, S, NB] fp16, parity per chunk
        xp = [[sb.tile([128, 12 * SN], F16, name=f"xp_{l}_{p}", tag=f"xp_{l}_{p}")
               for p in range(2)] for l in range(2)]
        xp4 = [[xp[l][p].rearrange("p (m t b) -> p m t b", m=12, b=NB)
                for p in range(2)] for l in range(2)]

        # h slabs [128, (S+1), 4, NB] fp16
        co = [sb.tile([128, (S + 1) * 4 * NB], F16, name=f"co_{l}", tag=f"co_{l}")
              for l in range(2)]
        co4 = [co[l].rearrange("p (t j b) -> p t j b", j=4, b=NB)
               for l in range(2)]
        # fixed-address h ping-pong (matmul operands must not use register
        # offsets -- each register AP costs an extra ~100ns Tensor op and
        # serializes the mm stream at ~171ns/pair instead of ~33ns)
        hfix = [[sb.tile([128, 4 * NB], F16, name=f"hfix_{l}_{p}",
                         tag=f"hfix_{l}_{p}") for p in range(2)]
                for l in range(2)]
        hfix4 = [[hfix[l][p].rearrange("p (j b) -> p j b", b=NB)
                  for p in range(2)] for l in range(2)]
        # fixed-address xp prefetch, fp16 (feeds the identity-matmul psum
        # injection and the t2 add; copy depends only on the chunk GEMM)
        xcur = [[sb.tile([128, 24], F16, name=f"xcur_{l}_{p}",
                         tag=f"xcur_{l}_{p}") for p in range(2)]
                for l in range(2)]

        # x input chunks (fp16), parity
        xin = [sb.tile([128, 4 * SN], F16, name=f"xin_{p}", tag=f"xin_{p}")
               for p in range(2)]
        # mid (gelu output) chunks [128, 4, S*NB] fp16, parity
        mid = [sb.tile([128, 4 * SN], F16, name=f"mid_{p}", tag=f"mid_{p}")
               for p in range(2)]
        mid4 = [mid[p].rearrange("p (j t b) -> p j t b", j=4, b=NB)
                for p in range(2)]

        # gate scratch: ONE set shared by both layers and all rounds. The
        # WAR hazards (each op's write waits the previous round/layer's last
        # reader) pin the Tile scheduler to the staggered round-robin order
        # on every engine queue -- without this it phase-groups ops by
        # dependency depth and a blocked op head-of-line-stalls ready work.
        def t(nm, w):
            return sb.tile([128, w], F32, name=nm, tag=nm)
        scratch = (t("sg", 16), t("t1", 8), t("t2", 8),
                   t("nn", 8), t("dd", 8), t("ee", 8))

        # gelu scratch (chunk-wide)
        erf_t = sb.tile([128, S * 4 * NB], F32, name="erf_t", tag="erf_t")
        gu = sb.tile([128, S * 4 * NB], F32, name="gu", tag="gu")

        nc.vector.memset(hfix[0][0][:], 0.0)
        nc.vector.memset(hfix[1][0][:], 0.0)

        # ---- helpers ------------------------------------------------------
        def dma_xin(k):
            t = xin[k % 2]
            for j in range(4):
                nc.sync.dma_start(t[:, j * SN:(j + 1) * SN],
                                  xT[j * 128:(j + 1) * 128, k * SN:(k + 1) * SN])

        def gemm(l, k, src_of_j):
            """xp[l][k%2] = wih_l^T @ src + bias (ACT copy adds bias)."""
            x4 = xp4[l][k % 2]
            for m in range(12):
                ps = psG[l][m % 2]
                for j in range(4):
                    nc.tensor.matmul(
                        ps[:, 0:SN],
                        wih_sb[l][:, (j * 12 + m) * 128:(j * 12 + m + 1) * 128],
                        src_of_j(j), start=(j == 0), stop=(j == 3))
                ps_v = ps[:, 0:SN].rearrange("p (t b) -> p t b", b=NB)
                nc.vector.tensor_scalar_add(x4[:, m, :, :], ps_v,
                                            bias_sb[l][:, m:m + 1])

        def step(l, i, par, co4l, x4):
            """One GRU step for layer l at time i, psum/h parity par."""
            P = psS[l][par]
            P4 = P[:, 0:24].rearrange("p (m b) -> p m b", b=NB)
            hc = hfix4[l][par]
            hnew = hfix[l][1 - par]
            xc = xcur[l][par]
            nc.vector.tensor_copy(xc[:], x4[:, :, bass.ds(i, 1), :])
            w = whh_sb[l]
            # xp_rz and b_hh_n enter the PSUM group via identity matmuls
            # (start=True pends the whole bank; every later mm accumulates)
            nc.tensor.matmul(P[:, 0:16], eye128[:, :], xc[:, 0:16],
                             start=True, stop=False)
            nc.tensor.matmul(P[:, 16:24], eye128[:, :], bhhn_sb[l][:, :],
                             start=False, stop=False)
            for m in range(12):
                for j in range(4):
                    nc.tensor.matmul(
                        P4[:, m, :],
                        w[:, (j * 12 + m) * 128:(j * 12 + m + 1) * 128],
                        hc[:, j, :], start=False,
                        stop=(m == 11 and j == 3))
            sgt, u1, u2, nnt, d, e = scratch
            nc.scalar.activation(sgt[:], P[:, 0:16], AF.Sigmoid)
            nc.vector.tensor_mul(u1[:], sgt[:, 0:8], P[:, 16:24])
            nc.vector.tensor_add(u2[:], u1[:], xc[:, 16:24])
            nc.scalar.activation(nnt[:], u2[:], AF.Tanh)
            nc.gpsimd.tensor_sub(d[:], hfix[l][par][:], nnt[:])
            nc.gpsimd.tensor_mul(e[:], sgt[:, 8:16], d[:])
            nc.gpsimd.tensor_add(hnew[:], nnt[:], e[:])
            nc.gpsimd.tensor_copy(co4l[:, bass.ds(i + 1, 1), :, :], hnew[:])

        def run_steps(layers):
            """layers: list of (l, xp-parity). Interleave steps in For_i."""
            with tc.For_i(0, S, U) as iv:
                for u in range(U):
                    i = iv + u
                    for (l, xpar) in layers:
                        step(l, i, u % 2, co4[l], xp4[l][xpar])

        def gelu(k):
            src = co4[0][:, 1:S + 1, :, :]
            nc.scalar.activation(erf_t[:], src, AF.Erf, scale=0.7071067811865476)
            nc.scalar.activation(gu[:], erf_t[:], AF.Identity,
                                 bias=half_t[:, 0:1], scale=0.5)
            m4 = mid4[k % 2]
            out_ap = m4.rearrange("p j t b -> p t j b")
            nc.vector.tensor_mul(out_ap, src, gu[:].rearrange(
                "p (t j b) -> p t j b", j=4, b=NB))

        def dma_y(k):
            nc.sync.dma_start(y4[:, k * S:(k + 1) * S, :, :],
                              co4[1][:, 1:S + 1, :, :])

        # ---- schedule -----------------------------------------------------
        # prologue: chunk 0 of L1 alone
        dma_xin(0)
        gemm(0, 0, lambda j: xin[0][:, j * SN:(j + 1) * SN])
        dma_xin(1)
        run_steps([(0, 0)])
        gelu(0)
        gemm(0, 1, lambda j: xin[1][:, j * SN:(j + 1) * SN])
        gemm(1, 0, lambda j, _k=0: mid4[0][:, j, :, :])

        for k in range(1, NCH):
            if k + 1 < NCH:
                dma_xin(k + 1)
            run_steps([(0, k % 2), (1, (k - 1) % 2)])
            dma_y(k - 1)
            gelu(k)
            if k + 1 < NCH:
                gemm(0, k + 1,
                     lambda j, _p=(k + 1) % 2: xin[_p][:, j * SN:(j + 1) * SN])
            gemm(1, k, lambda j, _p=k % 2: mid4[_p][:, j, :, :])

        # epilogue: last chunk of L2 alone
        run_steps([(1, (NCH - 1) % 2)])
        dma_y(NCH - 1)

    nc.compile()
    return nc


def _get_nc():
    if "nc" not in _CACHE:
        _CACHE["nc"] = _build()
    return _CACHE["nc"]


def _prep_core_inputs(x_slice, w_ih1, w_hh1, b_ih1, b_hh1,
                      w_ih2, w_hh2, b_ih2, b_hh2):
    def wstat(w):
        return np.ascontiguousarray(w.T).astype(np.float16)

    def bias12(b_ih, b_hh):
        b = b_ih.astype(np.float64).copy()
        b[:2 * H] += b_hh[:2 * H].astype(np.float64)
        # n gates: b_ih only (b_hh_n injected per step via bias matmuls)
        return np.ascontiguousarray(b.reshape(12, 128).T).astype(np.float32)

    def bhhn(b_hh):
        bn = b_hh[2 * H:].reshape(4, 128).T
        return np.ascontiguousarray(
            np.repeat(bn[:, :, None], NB, axis=2).reshape(128, 4 * NB)
        ).astype(np.float16)

    xT = np.ascontiguousarray(
        x_slice.transpose(2, 1, 0).reshape(512, T * NB)).astype(np.float16)
    return {
        "xT": xT,
        "wih1": wstat(w_ih1), "whh1": wstat(w_hh1),
        "bias1": bias12(b_ih1, b_hh1), "bhhn1": bhhn(b_hh1),
        "wih2": wstat(w_ih2), "whh2": wstat(w_hh2),
        "bias2": bias12(b_ih2, b_hh2), "bhhn2": bhhn(b_hh2),
        "eyed": np.eye(128, dtype=np.float16),
    }


def kernel(x, w_ih1, w_hh1, b_ih1, b_hh1, w_ih2, w_hh2, b_ih2, b_hh2):
    from concourse import bass_utils

    x = np.asarray(x, dtype=np.float32)
    args = [np.asarray(a, dtype=np.float32) for a in
            (w_ih1, w_hh1, b_ih1, b_hh1, w_ih2, w_hh2, b_ih2, b_hh2)]

    nc = _get_nc()
    in_maps = [
        _prep_core_inputs(x[c * NB:(c + 1) * NB], *args)
        for c in range(N_CORES)
    ]
    res = bass_utils.run_bass_kernel_spmd(nc, in_maps,
                                          core_ids=list(range(N_CORES)))
    parts = []
    for c in range(N_CORES):
        yf = res.results[c]["y"].astype(np.float32).reshape(128, T, 4, NB)
        parts.append(np.ascontiguousarray(
            yf.transpose(3, 1, 2, 0).reshape(NB, T, 512)))
    return np.concatenate(parts, axis=0)

